# revision 30
# baseline (speedup 1.0000x reference)
"""Trainium2 Bass kernel: masked multi-head attention (B=2, S=2048, D=512, H=8).

Sharding: batch x head-pair across 8 cores (core = b*4 + head_pair).
Each core computes, for its batch b and its 2 heads:
    q/k/v projections -> scores^T -> exp (mask folded in as per-partition
    bias on the ScalarE) -> attn@v with a ones-column appended to V (gives
    the softmax denominator for free) -> normalize -> partial out-proj.
The 4 per-batch partials are summed on the host (the "all-reduce"), then
bias bo is added.

Device layouts (per core):
  xTq/xTk/xTv  [D, S]    inputs pre-transposed on host (feature-major)
  q/k projT    [128, S]  2 local heads stacked on partitions (h0: 0-63)
  scores^T     [128k, q] per 128-wide key chunk; softmax mask depends only
                         on the key position -> per-partition ACT bias
  v_aug        [Sk, 130] per-head [ones | Wv_h] columns; attn@v output row
                         0 of each head block is the softmax denominator
  out          [512, S]  transposed partial output (host transposes back)

Program order interleaves phases so every engine stays busy:
  kproj -> qproj(first 1024) -> vproj -> attn(tp0,h0) -> attn(tp0,h1)
  -> qproj(second 1024) -> attn(tp1,h0) -> outproj(tp0) -> attn(tp1,h1)
  -> outproj(tp1).
attn@v lags the exp stream by 2 chunks so the PE never waits on the
ScalarE.  Softmax denominators are broadcast across partitions with the
Pool engine's partition_broadcast (the Pool engine is otherwise idle),
then recip+mul per 512 columns on the DVE so the chain pipelines into
the out-projection.

The kernel specializes on ceil(max(valid_lens)/128) key chunks: key
positions >= valid_len contribute exactly 0 attention weight (exp of a
large negative bias underflows to 0), so chunks beyond that bound are
skipped entirely.  This is derived from the runtime inputs, so the
kernel stays correct for any valid_lens.
"""

import math
import os
import sys

import numpy as np

for _p in ("/opt/trn_rl_repo",):
    if os.path.isdir(_p) and _p not in sys.path:
        sys.path.insert(0, _p)

import ml_dtypes

D_MODEL = 512
NUM_HEADS = 8
HEAD_DIM = 64
N_CORES = 8
LOCAL_F = 128          # features per core = 2 heads * 64
# per-head v block: [ones | 63 zero pad | v_h (64)] = 128 columns.  The
# ones column is FIRST so the softmax denominator lands on oT partition 0
# (the hardware partition_broadcast always reads partition 0), and the
# context rows occupy partitions 64:128 (DVE access patterns must start at
# a 32-aligned partition and not cross the 64-partition line mid-span).
VH = 128
VAUG = 2 * VH  # 256
MASK_NEG = -30000.0

# "bfloat16" or "float32r" (fp32 storage, full-rate matmul w/ reduced mult
# precision) or "float32" (exact, 4x slower matmuls)
DT_NAME = os.environ.get("ATTN_KERNEL_DT", "bfloat16")
TRACE = False

last_results = None  # BassKernelResults of the most recent run (for test.py)

_PROG_CACHE = {}


def _np_dt(name):
    return ml_dtypes.bfloat16 if name == "bfloat16" else np.float32


def _build(nch: int, seq: int, dt_name: str, qk_bias: bool, v_bias: bool):
    from contextlib import ExitStack

    import concourse.bass as bass  # noqa: F401
    import concourse.mybir as mybir
    import concourse.tile as tile
    from concourse import bacc

    DT = getattr(mybir.dt, dt_name)
    F32 = mybir.dt.float32
    F32R = mybir.dt.float32r
    EXP = mybir.ActivationFunctionType.Exp
    sk = nch * 128
    n_tp = seq // 1024
    assert seq % 1024 == 0
    lag = 2 if nch >= 3 else 1

    nc = bacc.Bacc("TRN2", target_bir_lowering=False, debug=False,
                   num_devices=N_CORES)

    def din(name, shape, dt=DT):
        return nc.dram_tensor(name, shape, dt, kind="ExternalInput").ap()

    xTq = din("xTq", [D_MODEL, seq])
    # xk/xv/wqkv come host-prearranged as [p, c, f] so staging is a single
    # straight DMA with multi-KB per-partition runs
    xTk = din("xTk", [128, 4, sk])
    xTv = din("xTv", [128, 4, sk])
    # wk separate (staged first - the k projection is the first consumer);
    # [wqT | wvT(128, packed)] column blocks in wqv
    wkT = din("wkT", [128, 4, LOCAL_F])
    WQV = 2 * LOCAL_F
    wqv = din("wqv", [128, 4, WQV])
    woT = din("woT", [LOCAL_F, D_MODEL])
    # f32 smalls: [bq | bk | bv_aug(VAUG) | maskb(nch)]
    NSM = 2 + VAUG + nch
    smalls_d = din("smalls", [128, NSM], F32)
    out_d = nc.dram_tensor("out", [D_MODEL, seq], DT,
                           kind="ExternalOutput").ap()

    with tile.TileContext(nc) as tc, ExitStack() as ctx:
        const = ctx.enter_context(tc.tile_pool(name="const", bufs=1))

        # ---- stage inputs into SBUF ----
        # weights/smalls on the scalar queue (parallel with the big input
        # loads on the sync HWDGE queue); inputs column-split so compute
        # can start before staging completes
        # wk/wqv/xk/xv are host-prearranged [p, c, f]: single straight
        # DMAs with 1-5KB per-partition runs (fast), on the scalar queue
        wk_sb = const.tile([128, 4, LOCAL_F], DT, tag="wk")
        nc.scalar.dma_start(out=wk_sb, in_=wkT)
        sm_sb = const.tile([128, NSM], F32, tag="sm")
        nc.scalar.dma_start(out=sm_sb, in_=smalls_d)
        wqv_sb = const.tile([128, 4, WQV], DT, tag="wqv")
        nc.scalar.dma_start(out=wqv_sb, in_=wqv)
        xv_sb = const.tile([128, 4, sk], DT, tag="xv")
        nc.scalar.dma_start(out=xv_sb, in_=xTv)
        wo_sb = const.tile([LOCAL_F, D_MODEL], DT, tag="wo")
        nc.scalar.dma_start(out=wo_sb, in_=woT)

        # xk whole (5KB runs); xq in 512-col slabs spanning all 4 d-chunks
        # (3D AP) so each qproj j0-chunk starts as soon as ITS slab lands
        xk_sb = const.tile([128, 4, sk], DT, tag="xk")
        nc.sync.dma_start(out=xk_sb, in_=xTk)
        xq_r = xTq.rearrange("(c p) f -> p c f", p=128)
        xq_sb = const.tile([128, 4, seq], DT, tag="xq")
        for j0 in range(0, seq, 512):
            nc.sync.dma_start(out=xq_sb[:, :, j0:j0 + 512],
                              in_=xq_r[:, :, j0:j0 + 512])

        bq_sb = sm_sb[:, 0:1]
        bk_sb = sm_sb[:, 1:2]
        bv_sb = sm_sb[:, 2:2 + VAUG]
        mb_sb = sm_sb[:, 2 + VAUG:2 + VAUG + nch]
        wq_of, wv_of = 0, LOCAL_F

        # ---- persistent SBUF tiles ----
        qT = const.tile([LOCAL_F, seq], DT, tag="qT")
        kT = const.tile([LOCAL_F, sk], DT, tag="kT")
        vaug = const.tile([128, nch, VAUG], DT, tag="vaug")
        stage = const.tile([VH, 2, seq], F32, tag="stage")
        rbs = [const.tile([128, seq], F32, tag="rb0", name="rb0"),
               const.tile([128, seq], F32, tag="rb1", name="rb1")]
        cn = const.tile([LOCAL_F, seq], DT, tag="cn")

        with (
            tc.tile_pool(name="psum", bufs=2, space="PSUM") as psum,
            tc.tile_pool(name="expp", bufs=4) as expp,
            tc.tile_pool(name="outp", bufs=2) as outp,
        ):
            # PE warm-up: dummy matmuls bridging until the first xk slab
            # lands, so the HAM clock-gate starts ramping before real work
            warm = const.tile([128, 512], DT, tag="warm")
            nc.vector.memset(warm, 0.0)
            wps = psum.tile([128, 512], F32, tag="pp", name="warm_ps")
            for _ in range(3):
                nc.tensor.matmul(wps, lhsT=warm[:, 0:128], rhs=warm,
                                 start=True, stop=True)

            def proj_qk(dst, w_sb, w_of, x_sb, b_sb, j0, width):
                w = min(512, width - j0)
                ps = psum.tile([128, 512], F32, tag="pp",
                               name=f"pp{w_of}_{j0}")
                for dc in range(4):
                    nc.tensor.matmul(
                        ps[:, :w],
                        lhsT=w_sb[:, dc, w_of:w_of + LOCAL_F],
                        rhs=x_sb[:, dc, j0:j0 + w],
                        start=(dc == 0), stop=(dc == 3),
                    )
                nc.vector.tensor_copy(out=dst[:, j0:j0 + w], in_=ps[:, :w])
                if qk_bias:
                    # separate op: TensorScalarPtr has 1 sync-wait slot
                    nc.vector.tensor_scalar_add(
                        out=dst[:, j0:j0 + w], in0=dst[:, j0:j0 + w],
                        scalar1=b_sb)

            def vproj():
                for h in range(2):
                    _ones = vaug[:, :, h * VH:h * VH + 1]
                    _pad = vaug[:, :, h * VH + 1:h * VH + 64]
                    if dt_name == "float32r":  # memset can't encode f32r
                        _ones = _ones.bitcast(F32)
                        _pad = _pad.bitcast(F32)
                    nc.vector.memset(_ones, 1.0)
                    nc.vector.memset(_pad, 0.0)
                for c in range(nch):
                    ps = psum.tile([128, LOCAL_F], F32, tag="pp",
                                   name=f"ppv{c}")
                    for dc in range(4):
                        nc.tensor.matmul(
                            ps,
                            lhsT=xv_sb[:, dc, c * 128:(c + 1) * 128],
                            rhs=wqv_sb[:, dc, wv_of:wv_of + LOCAL_F],
                            start=(dc == 0), stop=(dc == 3),
                        )
                    for h in range(2):
                        vs = slice(h * VH + 64, h * VH + VH)
                        nc.vector.tensor_copy(out=vaug[:, c, vs],
                                              in_=ps[:, h * 64:h * 64 + 64])
                        if v_bias:
                            nc.vector.tensor_add(
                                out=vaug[:, c, vs], in0=vaug[:, c, vs],
                                in1=bv_sb[:, vs])

            def attn_block(tp, h, fast):
                q0 = tp * 1024
                oT = {t: psum.tile([VH, 512], F32, tag="pp",
                                   name=f"oT{tp}{h}{t}") for t in range(2)}
                exs = [None] * nch

                def attn_v(c):
                    for t in range(2):
                        nc.tensor.matmul(
                            oT[t],
                            lhsT=vaug[:, c, h * VH:(h + 1) * VH],
                            rhs=exs[c][:, t * 512:(t + 1) * 512],
                            start=(c == 0), stop=(c == nch - 1),
                        )

                for c in range(nch):
                    sc = psum.tile([128, 1024], F32, tag="sc", bufs=3,
                                   name=f"sc{tp}{h}{c}")
                    for t in range(2):
                        nc.tensor.matmul(
                            sc[:, t * 512:(t + 1) * 512],
                            lhsT=kT[h * 64:(h + 1) * 64,
                                    c * 128:(c + 1) * 128],
                            rhs=qT[h * 64:(h + 1) * 64,
                                   q0 + t * 512:q0 + (t + 1) * 512],
                            start=True, stop=True,
                        )
                    ex = expp.tile([128, 1024], DT, tag="ex",
                                   name=f"ex{tp}{h}{c}")
                    nc.scalar.activation(
                        out=ex, in_=sc, func=EXP,
                        bias=mb_sb[:, c:c + 1],
                        scale=1.0 / math.sqrt(HEAD_DIM),
                    )
                    exs[c] = ex
                    # attn@v lags so the PE never waits on the exp stream
                    if c >= lag:
                        attn_v(c - lag)
                for c in range(max(nch - lag, 0), nch):
                    attn_v(c)

                # drain oT and normalize.  Denominator (row 0) is
                # broadcast across 64 partitions on the (otherwise idle)
                # Pool engine; recip+mul per 512 on the DVE so the chain
                # pipelines.  ACT does a drain copy only on the final
                # block (it is exp-saturated during attention).
                for t in range(2):
                    dst = stage[:, h, q0 + t * 512:q0 + (t + 1) * 512]
                    if fast and t == 1:
                        nc.scalar.copy(out=dst, in_=oT[t])
                    else:
                        nc.vector.tensor_copy(out=dst, in_=oT[t])
                # reciprocal the [1,512] denominator row in place at
                # partition 0 (DVE cost is free-size-bound, and base-64
                # unary ops are silently broken on HW), THEN broadcast the
                # reciprocal to all 128 partitions on the Pool engine; the
                # mul runs with both inputs at base 64.  Both recips are
                # issued first so the DVE does not idle during the
                # broadcast.
                rb = rbs[h]
                for t in range(2):
                    tsl = slice(q0 + t * 512, q0 + (t + 1) * 512)
                    nc.vector.reciprocal_approx_fast(
                        out=stage[0:1, h, tsl], in_=stage[0:1, h, tsl])
                for t in range(2):
                    tsl = slice(q0 + t * 512, q0 + (t + 1) * 512)
                    nc.gpsimd.partition_broadcast(
                        rb[:, tsl], stage[0:1, h, tsl])
                    nc.vector.tensor_mul(
                        out=cn[h * 64:(h + 1) * 64, tsl],
                        in0=stage[64:VH, h, tsl], in1=rb[64:128, tsl])

            def outproj(tp, tail):
                # sh-major at [128,512] grain: the sh0 column-half only
                # needs the first normalize muls, and each cast/DMA departs
                # as soon as its matmul drains.  Casts alternate DVE/ACT
                # (both outprojs run after the last exp, so ACT is free).
                q0 = tp * 1024
                for sh in range(2):
                    st0 = q0 + sh * 512
                    for odc in range(4):
                        fp = psum.tile([128, 512], F32, tag="sc", bufs=3,
                                       name=f"fp{tp}{sh}{odc}")
                        nc.tensor.matmul(
                            fp,
                            lhsT=wo_sb[:, odc * 128:(odc + 1) * 128],
                            rhs=cn[:, st0:st0 + 512],
                            start=True, stop=True)
                        ob = outp.tile([128, 512], DT, tag="ob",
                                       name=f"ob{tp}{sh}{odc}")
                        if odc % 2 == 1:
                            nc.scalar.copy(out=ob, in_=fp)
                        else:
                            nc.vector.tensor_copy(out=ob, in_=fp)
                        nc.sync.dma_start(
                            out=out_d[odc * 128:(odc + 1) * 128,
                                      st0:st0 + 512],
                            in_=ob)

            # ---- program order == desired engine order ----
            for j0 in range(0, sk, 512):
                proj_qk(kT, wk_sb, 0, xk_sb, bk_sb, j0, sk)
            for j0 in range(0, seq // 2, 512):
                proj_qk(qT, wqv_sb, wq_of, xq_sb, bq_sb, j0, seq)
            vproj()
            attn_block(0, 0, fast=False)
            attn_block(0, 1, fast=False)
            for j0 in range(seq // 2, seq, 512):
                proj_qk(qT, wqv_sb, wq_of, xq_sb, bq_sb, j0, seq)
            attn_block(1, 0, fast=False)
            attn_block(1, 1, fast=True)
            # outproj(0) here: its cn is long ready, so it fills the PE
            # while tp1's normalize chains drain (keeps the p-state up)
            outproj(0, tail=False)
            outproj(1, tail=True)

    nc.compile()
    return nc


def kernel(queries, keys, values, valid_lens, Wq, bq, Wk, bk, Wv, bv, Wo, bo):
    global last_results
    queries = np.asarray(queries, dtype=np.float32)
    keys = np.asarray(keys, dtype=np.float32)
    values = np.asarray(values, dtype=np.float32)
    valid_lens = np.asarray(valid_lens).astype(np.int64)
    Wq = np.asarray(Wq, dtype=np.float32)
    Wk = np.asarray(Wk, dtype=np.float32)
    Wv = np.asarray(Wv, dtype=np.float32)
    Wo = np.asarray(Wo, dtype=np.float32)
    bq = np.asarray(bq, dtype=np.float32)
    bk = np.asarray(bk, dtype=np.float32)
    bv = np.asarray(bv, dtype=np.float32)
    bo = np.asarray(bo, dtype=np.float32)

    B, S, D = queries.shape
    assert (B, D) == (2, D_MODEL) and S % 1024 == 0

    Lmax = int(min(max(int(valid_lens.max()), 1), S))
    nch = (Lmax + 127) // 128
    sk = nch * 128

    npdt = _np_dt(DT_NAME)
    qk_bias = bool(np.any(bq) or np.any(bk))
    v_bias = bool(np.any(bv))
    key = (nch, S, DT_NAME, qk_bias, v_bias)
    if key not in _PROG_CACHE:
        _PROG_CACHE[key] = _build(nch, S, DT_NAME, qk_bias, v_bias)
    nc = _PROG_CACHE[key]

    in_maps = []
    for core in range(N_CORES):
        b, hp = divmod(core, 4)
        L = int(valid_lens[b])
        fs = hp * LOCAL_F
        # wv weight block stays packed (128 cols); the vproj copies fan
        # it out into the padded [ones | pad63 | v_h] vaug layout
        wvT_aug = Wv[fs:fs + 128, :].T.copy()
        bv_aug = np.zeros((VAUG,), np.float32)
        bv_aug[0] = 1.0
        bv_aug[64:128] = bv[fs:fs + 64]
        bv_aug[VH] = 1.0
        bv_aug[VH + 64:VH + 128] = bv[fs + 64:fs + 128]
        if L == 0:
            mask = np.zeros((sk,), np.float32)  # result discarded on host
        else:
            mask = np.where(np.arange(sk) < L, 0.0, MASK_NEG).astype(np.float32)
        wkT = Wk[fs:fs + 128, :].T.copy()
        wqv = np.concatenate([Wq[fs:fs + 128, :].T, wvT_aug], axis=1)
        smalls = np.empty((128, 2 + VAUG + nch), np.float32)
        smalls[:, 0] = bq[fs:fs + 128]
        smalls[:, 1] = bk[fs:fs + 128]
        smalls[:, 2:2 + VAUG] = bv_aug
        smalls[:, 2 + VAUG:] = mask.reshape(nch, 128).T
        def pcf(a):  # [D, f] -> host-prearranged [128, 4, f]
            return np.ascontiguousarray(
                a.reshape(4, 128, a.shape[1]).transpose(1, 0, 2))

        in_maps.append({
            "xTq": np.ascontiguousarray(queries[b].T).astype(npdt),
            "xTk": pcf(keys[b, :sk].T).astype(npdt),
            "xTv": pcf(values[b, :sk].T).astype(npdt),
            "wkT": pcf(wkT).astype(npdt),
            "wqv": pcf(wqv).astype(npdt),
            "woT": np.ascontiguousarray(Wo[:, fs:fs + 128].T).astype(npdt),
            "smalls": smalls,
        })

    from concourse.bass_utils import run_bass_kernel_spmd
    res = run_bass_kernel_spmd(nc, in_maps, list(range(N_CORES)), trace=TRACE)
    last_results = res
    outs = [r["out"] for r in res.results]

    final = np.empty((B, S, D), np.float32)
    for b in range(B):
        acc = sum(outs[4 * b + i].astype(np.float32) for i in range(4))
        final[b] = acc.T + bo
        if int(valid_lens[b]) == 0:
            # uniform attention over all S positions (reference semantics
            # when every key is masked: softmax of a constant row)
            row = (values[b].mean(0) @ Wv.T + bv) @ Wo.T + bo
            final[b] = np.broadcast_to(row, (S, D))
    return final


# revision 31
# speedup vs baseline: 1.3040x; 1.3040x over previous
"""Trainium2 Bass kernel: masked multi-head attention (B=2, S=2048, D=512, H=8).

Sharding: batch x head-pair across 8 cores (core = b*4 + head_pair).
Each core computes, for its batch b and its 2 heads:
    q/k/v projections -> scores^T -> exp (mask folded in as per-partition
    bias on the ScalarE) -> attn@v with a ones-column appended to V (gives
    the softmax denominator for free) -> normalize -> partial out-proj.
The 4 per-batch partials are summed on the host (the "all-reduce"), then
bias bo is added.

Device layouts (per core):
  xTq/xTk/xTv  [D, S]    inputs pre-transposed on host (feature-major)
  q/k projT    [128, S]  2 local heads stacked on partitions (h0: 0-63)
  scores^T     [128k, q] per 128-wide key chunk; softmax mask depends only
                         on the key position -> per-partition ACT bias
  v_aug        [Sk, 130] per-head [ones | Wv_h] columns; attn@v output row
                         0 of each head block is the softmax denominator
  out          [512, S]  transposed partial output (host transposes back)

Program order interleaves phases so every engine stays busy:
  kproj -> qproj(first 1024) -> vproj -> attn(tp0,h0) -> attn(tp0,h1)
  -> qproj(second 1024) -> attn(tp1,h0) -> outproj(tp0) -> attn(tp1,h1)
  -> outproj(tp1).
attn@v lags the exp stream by 2 chunks so the PE never waits on the
ScalarE.  Softmax denominators are broadcast across partitions with the
Pool engine's partition_broadcast (the Pool engine is otherwise idle),
then recip+mul per 512 columns on the DVE so the chain pipelines into
the out-projection.

The kernel specializes on ceil(max(valid_lens)/128) key chunks: key
positions >= valid_len contribute exactly 0 attention weight (exp of a
large negative bias underflows to 0), so chunks beyond that bound are
skipped entirely.  This is derived from the runtime inputs, so the
kernel stays correct for any valid_lens.
"""

import math
import os
import sys

import numpy as np

for _p in ("/opt/trn_rl_repo",):
    if os.path.isdir(_p) and _p not in sys.path:
        sys.path.insert(0, _p)

import ml_dtypes

D_MODEL = 512
NUM_HEADS = 8
HEAD_DIM = 64
N_CORES = 8
LOCAL_F = 128          # features per core = 2 heads * 64
# per-head v block: [ones | 63 zero pad | v_h (64)] = 128 columns.  The
# ones column is FIRST so the softmax denominator lands on oT partition 0
# (the hardware partition_broadcast always reads partition 0), and the
# context rows occupy partitions 64:128 (DVE access patterns must start at
# a 32-aligned partition and not cross the 64-partition line mid-span).
VH = 128
VAUG = 2 * VH  # 256
MASK_NEG = -30000.0

# "bfloat16" or "float32r" (fp32 storage, full-rate matmul w/ reduced mult
# precision) or "float32" (exact, 4x slower matmuls)
DT_NAME = os.environ.get("ATTN_KERNEL_DT", "bfloat16")
TRACE = False

last_results = None  # BassKernelResults of the most recent run (for test.py)

_PROG_CACHE = {}


def _np_dt(name):
    return ml_dtypes.bfloat16 if name == "bfloat16" else np.float32


def _build(nch: int, seq: int, dt_name: str, qk_bias: bool, v_bias: bool):
    from contextlib import ExitStack

    import concourse.bass as bass  # noqa: F401
    import concourse.mybir as mybir
    import concourse.tile as tile
    from concourse import bacc

    DT = getattr(mybir.dt, dt_name)
    F32 = mybir.dt.float32
    F32R = mybir.dt.float32r
    EXP = mybir.ActivationFunctionType.Exp
    sk = nch * 128
    n_tp = seq // 1024
    assert seq % 1024 == 0
    lag = 2 if nch >= 3 else 1

    nc = bacc.Bacc("TRN2", target_bir_lowering=False, debug=False,
                   num_devices=N_CORES)

    def din(name, shape, dt=DT):
        return nc.dram_tensor(name, shape, dt, kind="ExternalInput").ap()

    xTq = din("xTq", [D_MODEL, seq])
    # xk/xv/wqkv come host-prearranged as [p, c, f] so staging is a single
    # straight DMA with multi-KB per-partition runs
    xTk = din("xTk", [128, 4, sk])
    xTv = din("xTv", [128, 4, sk])
    # wk separate (staged first - the k projection is the first consumer);
    # [wqT | wvT(128, packed)] column blocks in wqv
    wkT = din("wkT", [128, 4, LOCAL_F])
    WQV = 2 * LOCAL_F
    wqv = din("wqv", [128, 4, WQV])
    woT = din("woT", [LOCAL_F, D_MODEL])
    # f32 smalls: [bq | bk | bv_aug(VAUG) | maskb(nch)]
    NSM = 2 + VAUG + nch
    smalls_d = din("smalls", [128, NSM], F32)
    out_d = nc.dram_tensor("out", [D_MODEL, seq], DT,
                           kind="ExternalOutput").ap()

    with tile.TileContext(nc) as tc, ExitStack() as ctx:
        const = ctx.enter_context(tc.tile_pool(name="const", bufs=1))

        # ---- stage inputs into SBUF ----
        # weights/smalls on the scalar queue (parallel with the big input
        # loads on the sync HWDGE queue); inputs column-split so compute
        # can start before staging completes
        # wk/wqv/xk/xv are host-prearranged [p, c, f]: single straight
        # DMAs with 1-5KB per-partition runs (fast), on the scalar queue
        wk_sb = const.tile([128, 4, LOCAL_F], DT, tag="wk")
        nc.scalar.dma_start(out=wk_sb, in_=wkT)
        sm_sb = const.tile([128, NSM], F32, tag="sm")
        nc.scalar.dma_start(out=sm_sb, in_=smalls_d)
        wqv_sb = const.tile([128, 4, WQV], DT, tag="wqv")
        nc.scalar.dma_start(out=wqv_sb, in_=wqv)
        xv_sb = const.tile([128, 4, sk], DT, tag="xv")
        nc.scalar.dma_start(out=xv_sb, in_=xTv)
        wo_sb = const.tile([LOCAL_F, D_MODEL], DT, tag="wo")
        nc.scalar.dma_start(out=wo_sb, in_=woT)

        # xk whole (5KB runs); xq in 512-col slabs spanning all 4 d-chunks
        # (3D AP) so each qproj j0-chunk starts as soon as ITS slab lands
        xk_sb = const.tile([128, 4, sk], DT, tag="xk")
        nc.sync.dma_start(out=xk_sb, in_=xTk)
        xq_r = xTq.rearrange("(c p) f -> p c f", p=128)
        xq_sb = const.tile([128, 4, seq], DT, tag="xq")
        for j0 in range(0, seq, 512):
            nc.sync.dma_start(out=xq_sb[:, :, j0:j0 + 512],
                              in_=xq_r[:, :, j0:j0 + 512])

        bq_sb = sm_sb[:, 0:1]
        bk_sb = sm_sb[:, 1:2]
        bv_sb = sm_sb[:, 2:2 + VAUG]
        mb_sb = sm_sb[:, 2 + VAUG:2 + VAUG + nch]
        wq_of, wv_of = 0, LOCAL_F

        # ---- persistent SBUF tiles ----
        qT = const.tile([LOCAL_F, seq], DT, tag="qT")
        kT = const.tile([LOCAL_F, sk], DT, tag="kT")
        vaug = const.tile([128, nch, VAUG], DT, tag="vaug")
        stage = const.tile([VH, 2, seq], F32, tag="stage")
        rbs = [const.tile([128, seq], F32, tag="rb0", name="rb0"),
               const.tile([128, seq], F32, tag="rb1", name="rb1")]
        cn = const.tile([LOCAL_F, seq], DT, tag="cn")

        with (
            tc.tile_pool(name="psum", bufs=2, space="PSUM") as psum,
            tc.tile_pool(name="expp", bufs=4) as expp,
            tc.tile_pool(name="outp", bufs=2) as outp,
        ):
            # PE warm-up: dummy matmuls bridging until the first xk slab
            # lands, so the HAM clock-gate starts ramping before real work
            warm = const.tile([128, 512], DT, tag="warm")
            nc.vector.memset(warm, 0.0)
            wps = psum.tile([128, 512], F32, tag="pp", name="warm_ps")
            for _ in range(3):
                nc.tensor.matmul(wps, lhsT=warm[:, 0:128], rhs=warm,
                                 start=True, stop=True)

            def proj_qk(dst, w_sb, w_of, x_sb, b_sb, j0, width):
                w = min(512, width - j0)
                ps = psum.tile([128, 512], F32, tag="pp",
                               name=f"pp{w_of}_{j0}")
                for dc in range(4):
                    nc.tensor.matmul(
                        ps[:, :w],
                        lhsT=w_sb[:, dc, w_of:w_of + LOCAL_F],
                        rhs=x_sb[:, dc, j0:j0 + w],
                        start=(dc == 0), stop=(dc == 3),
                    )
                nc.vector.tensor_copy(out=dst[:, j0:j0 + w], in_=ps[:, :w])
                if qk_bias:
                    # separate op: TensorScalarPtr has 1 sync-wait slot
                    nc.vector.tensor_scalar_add(
                        out=dst[:, j0:j0 + w], in0=dst[:, j0:j0 + w],
                        scalar1=b_sb)

            def vproj():
                for h in range(2):
                    _ones = vaug[:, :, h * VH:h * VH + 1]
                    _pad = vaug[:, :, h * VH + 1:h * VH + 64]
                    if dt_name == "float32r":  # memset can't encode f32r
                        _ones = _ones.bitcast(F32)
                        _pad = _pad.bitcast(F32)
                    nc.vector.memset(_ones, 1.0)
                    nc.vector.memset(_pad, 0.0)
                for c in range(nch):
                    ps = psum.tile([128, LOCAL_F], F32, tag="pp",
                                   name=f"ppv{c}")
                    for dc in range(4):
                        nc.tensor.matmul(
                            ps,
                            lhsT=xv_sb[:, dc, c * 128:(c + 1) * 128],
                            rhs=wqv_sb[:, dc, wv_of:wv_of + LOCAL_F],
                            start=(dc == 0), stop=(dc == 3),
                        )
                    for h in range(2):
                        vs = slice(h * VH + 64, h * VH + VH)
                        nc.vector.tensor_copy(out=vaug[:, c, vs],
                                              in_=ps[:, h * 64:h * 64 + 64])
                        if v_bias:
                            nc.vector.tensor_add(
                                out=vaug[:, c, vs], in0=vaug[:, c, vs],
                                in1=bv_sb[:, vs])

            def attn_block(tp, h, fast):
                q0 = tp * 1024
                oT = {t: psum.tile([VH, 512], F32, tag="oT",
                                   name=f"oT{tp}{h}{t}") for t in range(2)}
                exs = [None] * nch

                def attn_v(c):
                    for t in range(2):
                        nc.tensor.matmul(
                            oT[t],
                            lhsT=vaug[:, c, h * VH:(h + 1) * VH],
                            rhs=exs[c][:, t * 512:(t + 1) * 512],
                            start=(c == 0), stop=(c == nch - 1),
                        )

                for c in range(nch):
                    sc = psum.tile([128, 1024], F32, tag="sc",
                                   name=f"sc{tp}{h}{c}")
                    for t in range(2):
                        nc.tensor.matmul(
                            sc[:, t * 512:(t + 1) * 512],
                            lhsT=kT[h * 64:(h + 1) * 64,
                                    c * 128:(c + 1) * 128],
                            rhs=qT[h * 64:(h + 1) * 64,
                                   q0 + t * 512:q0 + (t + 1) * 512],
                            start=True, stop=True,
                        )
                    ex = expp.tile([128, 1024], DT, tag="ex",
                                   name=f"ex{tp}{h}{c}")
                    nc.scalar.activation(
                        out=ex, in_=sc, func=EXP,
                        bias=mb_sb[:, c:c + 1],
                        scale=1.0 / math.sqrt(HEAD_DIM),
                    )
                    exs[c] = ex
                    # attn@v lags so the PE never waits on the exp stream
                    if c >= lag:
                        attn_v(c - lag)
                for c in range(max(nch - lag, 0), nch):
                    attn_v(c)

                # drain oT and normalize.  Denominator (row 0) is
                # broadcast across 64 partitions on the (otherwise idle)
                # Pool engine; recip+mul per 512 on the DVE so the chain
                # pipelines.  ACT does a drain copy only on the final
                # block (it is exp-saturated during attention).
                for t in range(2):
                    dst = stage[:, h, q0 + t * 512:q0 + (t + 1) * 512]
                    if fast and t == 1:
                        nc.scalar.copy(out=dst, in_=oT[t])
                    else:
                        nc.vector.tensor_copy(out=dst, in_=oT[t])
                # reciprocal the [1,512] denominator row in place at
                # partition 0 (DVE cost is free-size-bound, and base-64
                # unary ops are silently broken on HW), THEN broadcast the
                # reciprocal to all 128 partitions on the Pool engine; the
                # mul runs with both inputs at base 64.  Both recips are
                # issued first so the DVE does not idle during the
                # broadcast.
                rb = rbs[h]
                for t in range(2):
                    tsl = slice(q0 + t * 512, q0 + (t + 1) * 512)
                    nc.vector.reciprocal_approx_fast(
                        out=stage[0:1, h, tsl], in_=stage[0:1, h, tsl])
                for t in range(2):
                    tsl = slice(q0 + t * 512, q0 + (t + 1) * 512)
                    nc.gpsimd.partition_broadcast(
                        rb[:, tsl], stage[0:1, h, tsl])
                    nc.vector.tensor_mul(
                        out=cn[h * 64:(h + 1) * 64, tsl],
                        in0=stage[64:VH, h, tsl], in1=rb[64:128, tsl])

            def outproj(tp, tail):
                # per-odc [128,1024] tiles; casts alternate DVE/ACT (both
                # outprojs run after the last exp, so ACT is free); ob pool
                # is deep enough that casts never wait on the out DMAs
                q0 = tp * 1024
                for odc in range(4):
                    fp = psum.tile([128, 1024], F32, tag="sc",
                                   name=f"fp{tp}{odc}")
                    for sh in range(2):
                        st0 = q0 + sh * 512
                        nc.tensor.matmul(
                            fp[:, sh * 512:(sh + 1) * 512],
                            lhsT=wo_sb[:, odc * 128:(odc + 1) * 128],
                            rhs=cn[:, st0:st0 + 512],
                            start=True, stop=True)
                    ob = outp.tile([128, 1024], DT, tag="ob", bufs=4,
                                   name=f"ob{tp}{odc}")
                    if odc % 2 == 1:
                        nc.scalar.copy(out=ob, in_=fp)
                    else:
                        nc.vector.tensor_copy(out=ob, in_=fp)
                    nc.sync.dma_start(
                        out=out_d[odc * 128:(odc + 1) * 128, q0:q0 + 1024],
                        in_=ob)

            # ---- program order == desired engine order ----
            for j0 in range(0, sk, 512):
                proj_qk(kT, wk_sb, 0, xk_sb, bk_sb, j0, sk)
            for j0 in range(0, seq // 2, 512):
                proj_qk(qT, wqv_sb, wq_of, xq_sb, bq_sb, j0, seq)
            vproj()
            attn_block(0, 0, fast=False)
            attn_block(0, 1, fast=False)
            for j0 in range(seq // 2, seq, 512):
                proj_qk(qT, wqv_sb, wq_of, xq_sb, bq_sb, j0, seq)
            attn_block(1, 0, fast=False)
            attn_block(1, 1, fast=True)
            # outproj(0) here: its cn is long ready, so it fills the PE
            # while tp1's normalize chains drain (keeps the p-state up)
            outproj(0, tail=False)
            outproj(1, tail=True)

    nc.compile()
    return nc


def kernel(queries, keys, values, valid_lens, Wq, bq, Wk, bk, Wv, bv, Wo, bo):
    global last_results
    queries = np.asarray(queries, dtype=np.float32)
    keys = np.asarray(keys, dtype=np.float32)
    values = np.asarray(values, dtype=np.float32)
    valid_lens = np.asarray(valid_lens).astype(np.int64)
    Wq = np.asarray(Wq, dtype=np.float32)
    Wk = np.asarray(Wk, dtype=np.float32)
    Wv = np.asarray(Wv, dtype=np.float32)
    Wo = np.asarray(Wo, dtype=np.float32)
    bq = np.asarray(bq, dtype=np.float32)
    bk = np.asarray(bk, dtype=np.float32)
    bv = np.asarray(bv, dtype=np.float32)
    bo = np.asarray(bo, dtype=np.float32)

    B, S, D = queries.shape
    assert (B, D) == (2, D_MODEL) and S % 1024 == 0

    Lmax = int(min(max(int(valid_lens.max()), 1), S))
    nch = (Lmax + 127) // 128
    sk = nch * 128

    npdt = _np_dt(DT_NAME)
    qk_bias = bool(np.any(bq) or np.any(bk))
    v_bias = bool(np.any(bv))
    key = (nch, S, DT_NAME, qk_bias, v_bias)
    if key not in _PROG_CACHE:
        _PROG_CACHE[key] = _build(nch, S, DT_NAME, qk_bias, v_bias)
    nc = _PROG_CACHE[key]

    in_maps = []
    for core in range(N_CORES):
        b, hp = divmod(core, 4)
        L = int(valid_lens[b])
        fs = hp * LOCAL_F
        # wv weight block stays packed (128 cols); the vproj copies fan
        # it out into the padded [ones | pad63 | v_h] vaug layout
        wvT_aug = Wv[fs:fs + 128, :].T.copy()
        bv_aug = np.zeros((VAUG,), np.float32)
        bv_aug[0] = 1.0
        bv_aug[64:128] = bv[fs:fs + 64]
        bv_aug[VH] = 1.0
        bv_aug[VH + 64:VH + 128] = bv[fs + 64:fs + 128]
        if L == 0:
            mask = np.zeros((sk,), np.float32)  # result discarded on host
        else:
            mask = np.where(np.arange(sk) < L, 0.0, MASK_NEG).astype(np.float32)
        wkT = Wk[fs:fs + 128, :].T.copy()
        wqv = np.concatenate([Wq[fs:fs + 128, :].T, wvT_aug], axis=1)
        smalls = np.empty((128, 2 + VAUG + nch), np.float32)
        smalls[:, 0] = bq[fs:fs + 128]
        smalls[:, 1] = bk[fs:fs + 128]
        smalls[:, 2:2 + VAUG] = bv_aug
        smalls[:, 2 + VAUG:] = mask.reshape(nch, 128).T
        def pcf(a):  # [D, f] -> host-prearranged [128, 4, f]
            return np.ascontiguousarray(
                a.reshape(4, 128, a.shape[1]).transpose(1, 0, 2))

        in_maps.append({
            "xTq": np.ascontiguousarray(queries[b].T).astype(npdt),
            "xTk": pcf(keys[b, :sk].T).astype(npdt),
            "xTv": pcf(values[b, :sk].T).astype(npdt),
            "wkT": pcf(wkT).astype(npdt),
            "wqv": pcf(wqv).astype(npdt),
            "woT": np.ascontiguousarray(Wo[:, fs:fs + 128].T).astype(npdt),
            "smalls": smalls,
        })

    from concourse.bass_utils import run_bass_kernel_spmd
    res = run_bass_kernel_spmd(nc, in_maps, list(range(N_CORES)), trace=TRACE)
    last_results = res
    outs = [r["out"] for r in res.results]

    final = np.empty((B, S, D), np.float32)
    for b in range(B):
        acc = sum(outs[4 * b + i].astype(np.float32) for i in range(4))
        final[b] = acc.T + bo
        if int(valid_lens[b]) == 0:
            # uniform attention over all S positions (reference semantics
            # when every key is masked: softmax of a constant row)
            row = (values[b].mean(0) @ Wv.T + bv) @ Wo.T + bo
            final[b] = np.broadcast_to(row, (S, D))
    return final
